# revision 3
# baseline (speedup 1.0000x reference)
"""Distributed MHA kernel for Trainium2 (8 NeuronCores).

Problem: x,f:(2,2048,1024), W_qkv:(1024,3072), W_proj:(1024,1024), H=16 heads.
reference returns (out, attn2gcn) with
  attn2gcn = softmax(q k^T / sqrt(64)) v   (per head, concat over heads)
  out      = (attn2gcn + f) @ W_proj + b_proj

Sharding: tensor-parallel over heads — core c owns heads 2c, 2c+1 for both
batches (column block c*128 of the hidden dim).  Attention arithmetic is
bf16 matmuls with fp32 PSUM (the attn output's max-abs rel-err budget is
too tight for fp8 anywhere on that path); softmax stays fp32 on ACT.

The projection is split: out = f @ W_proj + attn2gcn @ W_proj + b.
  - f @ W_proj runs in bf16 against host-staged fT/W tiles.  It has no
    on-device dependencies at all, so its 8 m-passes are the PE filler
    that keeps the tensor engine hot through the collective windows
    (the HAM governor halves the PE clock after any idle window).
  - attn2gcn (avn) rides the AllToAll as fp8e4 (x32), quartering the
    collective bytes vs the old bf16 fused tensor, and the post-reshard
    contraction runs as fp8 DoubleRow matmuls: per head-parity half just
    2 instructions per m-tile (contraction 2x128 per instr at 0.5
    cycles/row), so the work after the last AllToAll is ~4k PE cycles
    instead of the old 33k.
  - scales: f, W_proj staged x32 (bf16), avn x32 (fp8) -> psum carries
    1024*out; the host divides by 1024 after gathering (pure numpy).

Per-core dataflow (as in the tuned baseline): qkvT = W-slice^T @ x^T in
head-packed tiles; batch-1 qkv emitted inside head 0's early attention
chunks as PE filler; attention per (head, batch) in scoresT layout,
software-pipelined so the PE runs ahead of the ACT exp; av^T accumulates
an all-ones 65th v column giving the softmax denominator for free;
normalization for chunk c-1 is emitted inside chunk c off the PE
critical path.  The head-0 AllToAll half fires mid-kernel; its DoubleRow
contraction and the f@W fillers interleave into later chunks; the head-1
half completes at the end with only the tiny DoubleRow tail behind it.
"""

import numpy as np

B, N, C, H, D = 2, 2048, 1024, 16, 64
BN = B * N
SCALE = D ** -0.5
N_CORES = 8
KT = C // 128      # 8 contraction tiles
NCH = BN // 512    # 8 qkv free chunks
PS = 32.0          # fp8/bf16 staging scale for the projection operands

_cached = None


def _build():
    from contextlib import ExitStack

    import concourse.mybir as mybir
    import concourse.tile as tile
    from concourse import bacc
    from concourse.masks import make_identity

    F32 = mybir.dt.float32
    BF16 = mybir.dt.bfloat16
    F8 = mybir.dt.float8e4
    EXP = mybir.ActivationFunctionType.Exp
    COPY = mybir.ActivationFunctionType.Copy
    DR = mybir.MatmulPerfMode.DoubleRow

    nc = bacc.Bacc("TRN2", target_bir_lowering=False, debug=False,
                   num_devices=N_CORES)

    xT_ext = nc.dram_tensor("xT", [C, BN], BF16, kind="ExternalInput").ap()
    wkqv_ext = nc.dram_tensor("wkqv", [C, 384], BF16, kind="ExternalInput").ap()
    fT_ext = nc.dram_tensor("fT", [C, 512], BF16, kind="ExternalInput").ap()
    wproj_ext = nc.dram_tensor("wproj", [C, C], BF16, kind="ExternalInput").ap()
    wp8_ext = [nc.dram_tensor(f"wp8_{h}", [128, 4 * C], mybir.dt.float8e4,
                              kind="ExternalInput").ap() for h in range(2)]
    bprojT_ext = nc.dram_tensor("bprojT", [128, 8], F32, kind="ExternalInput").ap()
    attn_t_ext = nc.dram_tensor("attn_t", [128, BN], BF16, kind="ExternalOutput").ap()
    out_t_ext = nc.dram_tensor("out_t", [C, 512], BF16, kind="ExternalOutput").ap()

    groups = [list(range(N_CORES))]

    with tile.TileContext(nc) as tc:
        with ExitStack() as octx:
            pp = octx.enter_context(tc.tile_pool(name="persist", bufs=1))
            kqp = octx.enter_context(tc.tile_pool(name="kq", bufs=1))
            vap = octx.enter_context(tc.tile_pool(name="vaug", bufs=1))
            vtp = octx.enter_context(tc.tile_pool(name="vt", bufs=1))
            wqp = octx.enter_context(tc.tile_pool(name="wq", bufs=1))
            xsp = octx.enter_context(tc.tile_pool(name="xs", bufs=16))
            dram = octx.enter_context(
                tc.tile_pool(name="dram", bufs=1, space="DRAM"))
            # qkv weights first in the DMA queues — they gate the first matmul
            wq_sb = []
            for k in range(KT):
                w = wqp.tile([128, 384], BF16, name=f"wq{k}")
                nc.sync.dma_start(w[:], wkqv_ext[k * 128:(k + 1) * 128, :])
                wq_sb.append(w)

            ident = pp.tile([128, 128], BF16)
            make_identity(nc, ident[:])
            ones64b = pp.tile([1, 64], BF16)
            nc.vector.memset(ones64b[:], 1.0)

            kT = kqp.tile([128, BN], BF16, name="kT")
            qT = kqp.tile([128, BN], BF16, name="qT")
            vT = vtp.tile([128, BN], BF16, name="vT")
            mtiles = [kT, qT, vT]
            v_aug = [[vap.tile([128, 65], BF16, name=f"va{h}_{j}")
                      for j in range(32)] for h in range(2)]

            a2a_in = [dram.tile([512, 512], F8, name=f"a2ain{hh}")
                      for hh in range(2)]
            a2a_out = [dram.tile([512, 512], F8, name=f"a2aout{hh}")
                       for hh in range(2)]

            def qkv_transposes(nch, tpool, ttag):
                for j in range(4 * nch, 4 * nch + 4):
                    tps = tpool.tile([128, 128], BF16, name="tps", tag=ttag)
                    nc.tensor.transpose(
                        tps[:], vT[:, j * 128:(j + 1) * 128], ident[:])
                    for h in range(2):
                        nc.vector.tensor_copy(
                            v_aug[h][j][:, 0:64], tps[:, h * 64:(h + 1) * 64])
                        nc.vector.memset(v_aug[h][j][:, 64:65], 1.0)

            def qkv_xs(nch):
                xs_t = []
                for k in range(KT):
                    xs = xsp.tile([128, 512], BF16, name="xs", tag="xs")
                    nc.sync.dma_start(
                        xs[:], xT_ext[k * 128:(k + 1) * 128,
                                      nch * 512:(nch + 1) * 512])
                    xs_t.append(xs)
                return xs_t

            # ------------- phase Q: qkv for the first two chunks -------------
            with ExitStack() as qctx:
                qps = qctx.enter_context(
                    tc.tile_pool(name="qkv_ps", bufs=1, space="PSUM"))
                trp = qctx.enter_context(
                    tc.tile_pool(name="tr_ps", bufs=2, space="PSUM"))
                for nch in range(2):
                    xs_t = qkv_xs(nch)
                    psums = [qps.tile([128, 512], F32, name=f"qps{m}",
                                      tag=f"qps{m}") for m in range(3)]
                    for k in range(KT):
                        for m in range(3):
                            nc.tensor.matmul(
                                psums[m][:],
                                wq_sb[k][:, m * 128:(m + 1) * 128],
                                xs_t[k][:], start=(k == 0), stop=(k == KT - 1))
                    for m in range(3):
                        nc.vector.tensor_copy(
                            mtiles[m][:, nch * 512:(nch + 1) * 512],
                            psums[m][:])
                    qkv_transposes(nch, trp, "tps")

            # loads needed by the f@W fillers / normalization, queued behind
            # the phase-Q traffic but ahead of the deferred-qkv x chunks
            bias_sb = pp.tile([128, 8], F32)
            nc.sync.dma_start(bias_sb[:], bprojT_ext[:])
            fT_sb = []
            for t in range(KT):
                ft = pp.tile([128, 512], BF16, name=f"fTsb{t}")
                nc.sync.dma_start(ft[:], fT_ext[t * 128:(t + 1) * 128, :])
                fT_sb.append(ft)

            # ---------------- phase A: attention + fillers ----------------
            with ExitStack() as actx:
                expp = actx.enter_context(tc.tile_pool(name="exp", bufs=4))
                avup = actx.enter_context(tc.tile_pool(name="avu", bufs=3))
                normp = actx.enter_context(tc.tile_pool(name="norm", bufs=2))
                avnp = actx.enter_context(tc.tile_pool(name="avn", bufs=2))
                avn8p = actx.enter_context(tc.tile_pool(name="avn8", bufs=2))
                oaccp = actx.enter_context(tc.tile_pool(name="oacc", bufs=1))
                wpp = actx.enter_context(tc.tile_pool(name="wp", bufs=1))
                wp8p = actx.enter_context(tc.tile_pool(name="wp8", bufs=1))
                rhs8p = actx.enter_context(tc.tile_pool(name="rhs8", bufs=1))
                sps = actx.enter_context(
                    tc.tile_pool(name="scores_ps", bufs=2, space="PSUM"))
                avps = actx.enter_context(
                    tc.tile_pool(name="av_ps", bufs=1, space="PSUM"))
                bcps = actx.enter_context(
                    tc.tile_pool(name="bc_ps", bufs=1, space="PSUM"))
                pjps = actx.enter_context(
                    tc.tile_pool(name="pj_ps", bufs=1, space="PSUM"))
                out_acc = [oaccp.tile([128, 512], F32, name=f"oacc{m}")
                           for m in range(8)]
                # prefetch projection weights during early attention
                wp_sb = []
                for t in range(KT):
                    w = wpp.tile([128, C], BF16, name=f"wp_{t}")
                    nc.sync.dma_start(w[:], wproj_ext[t * 128:(t + 1) * 128, :])
                    wp_sb.append(w)
                wp8_sb = []
                for hh in range(2):
                    w8 = wp8p.tile([128, 4, C], F8, name=f"wp8_{hh}")
                    nc.sync.dma_start(w8[:], wp8_ext[hh][:])
                    wp8_sb.append(w8)

                qkv_work = [(nch, m) for nch in range(2, 8) for m in range(3)]
                fw_work = list(range(8))
                pe_work = []   # 4-m-tile DoubleRow units for the h0 half
                rhs8_sb = {}

                qkv_xs_cache = {}

                def qkv_deferred_unit(unit):
                    """One m-pass of a deferred qkv chunk — small PE
                    filler emitted inside head 0's early attention chunks
                    (single pj psum slot)."""
                    nch, m = unit
                    if nch not in qkv_xs_cache:
                        qkv_xs_cache[nch] = qkv_xs(nch)
                    xs_t = qkv_xs_cache[nch]
                    pjt = pjps.tile([128, 512], F32, name="qkvd", tag="pj")
                    for k in range(KT):
                        nc.tensor.matmul(
                            pjt[:], wq_sb[k][:, m * 128:(m + 1) * 128],
                            xs_t[k][:], start=(k == 0), stop=(k == KT - 1))
                    nc.vector.tensor_copy(
                        mtiles[m][:, nch * 512:(nch + 1) * 512], pjt[:])
                    if m == 2:
                        qkv_transposes(nch, bcps, "bc")
                        del qkv_xs_cache[nch]

                def fw_m(m):
                    """f @ W_proj m-pass: dependency-free bf16 PE filler."""
                    pj = pjps.tile([128, 512], F32, name="pjf", tag="pj")
                    for t in range(KT):
                        nc.tensor.matmul(
                            pj[:], wp_sb[t][:, m * 128:(m + 1) * 128],
                            fT_sb[t][:], start=(t == 0), stop=(t == KT - 1))
                    nc.vector.tensor_scalar_add(
                        out_acc[m][:], pj[:], bias_sb[:, m:m + 1])

                def load_rhs8(hh):
                    """Stack the 8 received [64,512] fp8 tiles of half hh
                    into two DoubleRow rhs tiles [128, 2, 512]."""
                    tiles = []
                    for u in range(2):
                        r = rhs8p.tile([128, 2, 512], F8, name=f"r8_{hh}{u}")
                        for i in range(2):
                            s = 2 * u + i
                            nc.sync.dma_start(
                                r[0:64, i, :],
                                a2a_out[hh][(2 * s) * 64:(2 * s + 1) * 64, :])
                            nc.sync.dma_start(
                                r[64:128, i, :],
                                a2a_out[hh][(2 * s + 1) * 64:(2 * s + 2) * 64, :])
                        tiles.append(r)
                    rhs8_sb[hh] = tiles

                def proj8_unit(hh, ms, final):
                    """DoubleRow avn@W for head-parity half hh, m-tiles ms."""
                    if hh not in rhs8_sb:
                        load_rhs8(hh)
                    r8 = rhs8_sb[hh]
                    w8 = wp8_sb[hh]
                    for m in ms:
                        pj = pjps.tile([128, 512], F32, name="pj8", tag="pj")
                        for u in range(2):
                            nc.tensor.matmul(
                                pj[:], w8[:, 2 * u:2 * u + 2,
                                          m * 128:(m + 1) * 128],
                                r8[u][:], start=(u == 0), stop=(u == 1),
                                perf_mode=DR)
                        if final:
                            ot = avnp.tile([128, 512], BF16, name="ot",
                                           tag="ot")
                            nc.vector.tensor_tensor(
                                ot[:], pj[:], out_acc[m][:],
                                mybir.AluOpType.add)
                            nc.sync.dma_start(
                                out_t_ext[m * 128:(m + 1) * 128, :], ot[:])
                        else:
                            nc.vector.tensor_tensor(
                                out_acc[m][:], pj[:], out_acc[m][:],
                                mybir.AluOpType.add)

                def norm_pre(avu):
                    """1/denom chain — latency starts at chunk end."""
                    dn = normp.tile([1, 1024], F32, name="dn", tag="dn")
                    nc.sync.dma_start(dn[:], avu[64:65, :])
                    dninv = normp.tile([1, 1024], F32, name="dninv",
                                       tag="dninv")
                    nc.vector.reciprocal_approx_fast(dninv[:], dn[:])
                    dinvb = normp.tile([1, 1024], BF16, name="dinvb",
                                       tag="dinvb")
                    nc.scalar.activation(dinvb[:], dninv[:], COPY)
                    return dinvb

                def norm_chunk(h, b, ch, avu, dinvb):
                    """avn = avu[0:64]/avu[64]; attn_t out; fp8 a2a staging."""
                    po = h * 64
                    cs = b * 2048 + ch * 1024
                    avn = avnp.tile([64, 1024], BF16, name="avn", tag="avn")
                    for s in range(2):
                        bc = bcps.tile([64, 512], F32, name="bc", tag="bc")
                        nc.tensor.matmul(bc[:], ones64b[:],
                                         dinvb[:, s * 512:(s + 1) * 512],
                                         start=True, stop=True)
                        nc.vector.tensor_tensor(
                            avn[:, s * 512:(s + 1) * 512],
                            avu[0:64, s * 512:(s + 1) * 512],
                            bc[:], mybir.AluOpType.mult)
                    nc.sync.dma_start(
                        attn_t_ext[po:po + 64, cs:cs + 1024], avn[:])
                    avn8 = avn8p.tile([64, 1024], F8, name="avn8", tag="avn8")
                    nc.vector.tensor_scalar_mul(avn8[:], avn[:], PS)
                    for j in (cs // 512, cs // 512 + 1):
                        nc.sync.dma_start(
                            a2a_in[h][j * 64:(j + 1) * 64, :],
                            avn8[:, (j * 512 - cs):(j * 512 - cs) + 512])

                def mm1_kj(h, b, cs, kj, scores_q):
                    po = h * 64
                    jt = b * 16 + kj
                    sc = sps.tile([128, 1024], F32, name="scores", tag="sc")
                    for s in range(2):
                        nc.tensor.matmul(
                            sc[:, s * 512:(s + 1) * 512],
                            kT[po:po + 64, jt * 128:(jt + 1) * 128],
                            qT[po:po + 64, cs + s * 512:cs + (s + 1) * 512],
                            start=True, stop=True)
                    scores_q[kj] = sc

                def fire_a2a(h):
                    nc.gpsimd.collective_compute(
                        "AllToAll", mybir.AluOpType.bypass,
                        replica_groups=groups,
                        ins=[a2a_in[h].opt()], outs=[a2a_out[h].opt()])
                    if h == 0:
                        pe_work.extend([(0, range(0, 4)), (0, range(4, 8))])

                def do_chunk(ci, h, b, ch, pending):
                    cs = b * 2048 + ch * 1024
                    av = avps.tile([128, 1024], F32, name="av", tag="av")
                    scores_q = {}
                    mm1_kj(h, b, cs, 0, scores_q)
                    for kj in range(16):
                        if kj + 1 < 16:
                            mm1_kj(h, b, cs, kj + 1, scores_q)
                        if kj == 4 and pending is not None:
                            ph = pending[0]
                            norm_chunk(*pending)
                            pending = None
                            if ph != h:
                                # that was the previous head's last chunk:
                                # its AllToAll half can fire now
                                fire_a2a(ph)
                        if ci < 3 and kj in (1, 3, 5) and qkv_work:
                            qkv_deferred_unit(qkv_work.pop(0))
                            if qkv_work:
                                qkv_deferred_unit(qkv_work.pop(0))
                        if ci >= 3 and kj in (1, 5, 9) and fw_work:
                            fw_m(fw_work.pop(0))
                        if ci >= 6 and kj in (11, 13) and pe_work:
                            proj8_unit(*pe_work.pop(0), final=False)
                        sc = scores_q.pop(kj)
                        ex = expp.tile([128, 1024], BF16, name="ex", tag="ex")
                        nc.scalar.activation(ex[:], sc[:], EXP, scale=SCALE)
                        jt = b * 16 + kj
                        for s in range(2):
                            nc.tensor.matmul(
                                av[0:65, s * 512:(s + 1) * 512],
                                v_aug[h][jt][:],
                                ex[:, s * 512:(s + 1) * 512],
                                start=(kj == 0), stop=(kj == 15))
                    avu = avup.tile([65, 1024], F32, name="avu", tag="avu")
                    nc.vector.tensor_copy(avu[:], av[0:65, :])
                    return (h, b, ch, avu, norm_pre(avu))

                pending = None
                ci = 0
                for h in range(2):
                    for b in range(2):
                        for ch in range(2):
                            pending = do_chunk(ci, h, b, ch, pending)
                            ci += 1
                # final chunk of head 1, then its collective
                norm_chunk(*pending)
                fire_a2a(1)

                # any h0 DoubleRow units not consumed as fillers
                while pe_work:
                    proj8_unit(*pe_work.pop(0), final=False)
                while fw_work:
                    fw_m(fw_work.pop(0))

                # tail: head-parity-1 DoubleRow contraction + output
                proj8_unit(1, range(8), final=True)

    nc.compile()
    return nc


def kernel(x, f, W_qkv, W_proj, b_proj):
    import ml_dtypes
    from concourse.bass_utils import run_bass_kernel_spmd

    global _cached
    if _cached is None:
        _cached = _build()
    nc = _cached

    BF = ml_dtypes.bfloat16
    F8 = ml_dtypes.float8_e4m3
    x = np.ascontiguousarray(np.asarray(x, dtype=np.float32))
    f = np.ascontiguousarray(np.asarray(f, dtype=np.float32))
    W_qkv = np.asarray(W_qkv, dtype=np.float32)
    W_proj = np.asarray(W_proj, dtype=np.float32)
    b_proj = np.asarray(b_proj, dtype=np.float32)

    Wq, Wk, Wv = W_qkv[:, 0:C], W_qkv[:, C:2 * C], W_qkv[:, 2 * C:3 * C]
    xT = np.ascontiguousarray(x.reshape(BN, C).T.astype(BF))
    fT = (f.reshape(BN, C).T * PS).astype(BF)
    wproj_b = np.ascontiguousarray((W_proj * PS).astype(BF))
    bprojT = np.ascontiguousarray((b_proj * PS * PS).reshape(8, 128).T)

    # DoubleRow fp8 W_proj halves: wp8_h[p, s, m] = (W_proj*PS)[row, m] with
    # row = head*64 + p%64, head = 4s + h + 2*(p//64)
    Wp32 = W_proj * PS
    p = np.arange(128)
    wp8 = []
    for h in range(2):
        rows = np.empty((128, 4), np.int64)
        for s in range(4):
            head = 4 * s + h + 2 * (p // 64)
            rows[:, s] = head * 64 + (p % 64)
        wp8.append(np.ascontiguousarray(
            Wp32[rows, :].astype(F8).reshape(128, 4 * C)))

    in_maps = []
    for c in range(N_CORES):
        cols = slice(c * 128, (c + 1) * 128)     # heads 2c, 2c+1
        wkqv = np.ascontiguousarray(np.concatenate(
            [Wk[:, cols], Wq[:, cols], Wv[:, cols]], axis=1).astype(BF))
        in_maps.append({
            "xT": xT,
            "wkqv": wkqv,
            "fT": np.ascontiguousarray(fT[:, c * 512:(c + 1) * 512]),
            "wproj": wproj_b,
            "wp8_0": wp8[0],
            "wp8_1": wp8[1],
            "bprojT": bprojT,
        })

    res = run_bass_kernel_spmd(nc, in_maps, core_ids=list(range(N_CORES)))

    attn = np.empty((BN, C), dtype=np.float32)
    out = np.empty((BN, C), dtype=np.float32)
    for c in range(N_CORES):
        r = res.results[c]
        attn[:, c * 128:(c + 1) * 128] = r["attn_t"].T.astype(np.float32)
        out[c * 512:(c + 1) * 512, :] = \
            r["out_t"].T.astype(np.float32) / (PS * PS)
    return out.reshape(B, N, C), attn.reshape(B, N, C)


# revision 9
# speedup vs baseline: 1.3183x; 1.3183x over previous
"""Distributed MHA kernel for Trainium2 (8 NeuronCores).

Problem: x,f:(2,2048,1024), W_qkv:(1024,3072), W_proj:(1024,1024), H=16 heads.
reference returns (out, attn2gcn) with
  attn2gcn = softmax(q k^T / sqrt(64)) v   (per head, concat over heads)
  out      = (attn2gcn + f) @ W_proj + b_proj

Sharding: tensor-parallel over heads — core c owns heads 2c, 2c+1 for both
batches (column block c*128 of the hidden dim).  Attention arithmetic is
bf16 matmuls with fp32 PSUM (the attn output's max-abs rel-err budget is
too tight for fp8 anywhere on that path); softmax stays fp32 on ACT.

The projection is split: out = f @ W_proj + attn2gcn @ W_proj + b.
  - f @ W_proj runs in bf16 against host-staged fT/W tiles.  It has no
    on-device dependencies at all, so its 8 m-passes are the PE filler
    that keeps the tensor engine hot through the collective windows
    (the HAM governor halves the PE clock after any idle window).
  - attn2gcn (avn) rides the AllToAll as bf16 (x32; fp8 collectives
    measured pathologically slow), is cast to fp8e4 after the reshard,
    and the contraction runs as fp8 DoubleRow matmuls: per head-parity
    half just 2 instructions per m-tile (contraction 2x128 per instr at
    0.5 cycles/row), so the work after the last AllToAll is ~4k PE
    cycles instead of the old 33k.
  - scales: f, W_proj staged x32 (bf16), avn x32 (fp8) -> psum carries
    1024*out; the host divides by 1024 after gathering (pure numpy).

Per-core dataflow (as in the tuned baseline): qkvT = W-slice^T @ x^T in
head-packed tiles; batch-1 qkv emitted inside head 0's early attention
chunks as PE filler; attention per (head, batch) in scoresT layout,
software-pipelined so the PE runs ahead of the ACT exp; av^T accumulates
an all-ones 65th v column giving the softmax denominator for free;
normalization for chunk c-1 is emitted inside chunk c off the PE
critical path.  The head-0 AllToAll half fires mid-kernel; its DoubleRow
contraction and the f@W fillers interleave into later chunks; the head-1
half completes at the end with only the tiny DoubleRow tail behind it.
"""

import numpy as np

B, N, C, H, D = 2, 2048, 1024, 16, 64
BN = B * N
SCALE = D ** -0.5
N_CORES = 8
KT = C // 128      # 8 contraction tiles
NCH = BN // 512    # 8 qkv free chunks
PS = 32.0          # fp8/bf16 staging scale for the projection operands

_cached = None


def _build():
    from contextlib import ExitStack

    import concourse.mybir as mybir
    import concourse.tile as tile
    from concourse import bacc
    from concourse.masks import make_identity

    F32 = mybir.dt.float32
    BF16 = mybir.dt.bfloat16
    F8 = mybir.dt.float8e4
    EXP = mybir.ActivationFunctionType.Exp
    COPY = mybir.ActivationFunctionType.Copy
    DR = mybir.MatmulPerfMode.DoubleRow

    nc = bacc.Bacc("TRN2", target_bir_lowering=False, debug=False,
                   num_devices=N_CORES)

    xT_ext = nc.dram_tensor("xT", [C, BN], BF16, kind="ExternalInput").ap()
    wkqv_ext = nc.dram_tensor("wkqv", [C, 384], BF16, kind="ExternalInput").ap()
    fT_ext = nc.dram_tensor("fT", [C, 512], BF16, kind="ExternalInput").ap()
    wproj_ext = nc.dram_tensor("wproj", [C, C], BF16, kind="ExternalInput").ap()
    wp8_ext = [nc.dram_tensor(f"wp8_{h}", [128, 4 * C], mybir.dt.float8e4,
                              kind="ExternalInput").ap() for h in range(2)]
    bprojT_ext = nc.dram_tensor("bprojT", [128, 8], F32, kind="ExternalInput").ap()
    attn_t_ext = nc.dram_tensor("attn_t", [128, BN], BF16, kind="ExternalOutput").ap()
    out_t_ext = nc.dram_tensor("out_t", [C, 512], BF16, kind="ExternalOutput").ap()

    groups = [list(range(N_CORES))]

    with tile.TileContext(nc) as tc:
        with ExitStack() as octx:
            pp = octx.enter_context(tc.tile_pool(name="persist", bufs=1))
            kqp = octx.enter_context(tc.tile_pool(name="kq", bufs=1))
            vap = octx.enter_context(tc.tile_pool(name="vaug", bufs=1))
            vtp = octx.enter_context(tc.tile_pool(name="vt", bufs=1))
            wqp = octx.enter_context(tc.tile_pool(name="wq", bufs=1))
            xsp = octx.enter_context(tc.tile_pool(name="xs", bufs=16))
            dram = octx.enter_context(
                tc.tile_pool(name="dram", bufs=1, space="DRAM"))
            # qkv weights first in the DMA queues — they gate the first matmul
            wq_sb = []
            for k in range(KT):
                w = wqp.tile([128, 384], BF16, name=f"wq{k}")
                nc.sync.dma_start(w[:], wkqv_ext[k * 128:(k + 1) * 128, :])
                wq_sb.append(w)

            ident = pp.tile([128, 128], BF16)
            make_identity(nc, ident[:])
            ones64b = pp.tile([1, 64], BF16)
            nc.vector.memset(ones64b[:], 1.0)

            kT = kqp.tile([128, BN], BF16, name="kT")
            qT = kqp.tile([128, BN], BF16, name="qT")
            vT = vtp.tile([128, BN], BF16, name="vT")
            mtiles = [kT, qT, vT]
            v_aug = [[vap.tile([128, 65], BF16, name=f"va{h}_{j}")
                      for j in range(32)] for h in range(2)]

            a2a_in = [dram.tile([512, 512], BF16, name=f"a2ain{hh}")
                      for hh in range(2)]
            a2a_out = [dram.tile([512, 512], BF16, name=f"a2aout{hh}")
                       for hh in range(2)]

            def qkv_transposes(nch, tpool, ttag):
                for j in range(4 * nch, 4 * nch + 4):
                    tps = tpool.tile([128, 128], BF16, name="tps", tag=ttag)
                    nc.tensor.transpose(
                        tps[:], vT[:, j * 128:(j + 1) * 128], ident[:])
                    for h in range(2):
                        nc.vector.tensor_copy(
                            v_aug[h][j][:, 0:64], tps[:, h * 64:(h + 1) * 64])
                        nc.vector.memset(v_aug[h][j][:, 64:65], 1.0)

            def qkv_xs(nch):
                xs_t = []
                for k in range(KT):
                    xs = xsp.tile([128, 512], BF16, name="xs", tag="xs")
                    nc.sync.dma_start(
                        xs[:], xT_ext[k * 128:(k + 1) * 128,
                                      nch * 512:(nch + 1) * 512])
                    xs_t.append(xs)
                return xs_t

            # ------------- phase Q: qkv for the first two chunks -------------
            with ExitStack() as qctx:
                qps = qctx.enter_context(
                    tc.tile_pool(name="qkv_ps", bufs=1, space="PSUM"))
                trp = qctx.enter_context(
                    tc.tile_pool(name="tr_ps", bufs=2, space="PSUM"))
                for nch in range(2):
                    xs_t = qkv_xs(nch)
                    psums = [qps.tile([128, 512], F32, name=f"qps{m}",
                                      tag=f"qps{m}") for m in range(3)]
                    for k in range(KT):
                        for m in range(3):
                            nc.tensor.matmul(
                                psums[m][:],
                                wq_sb[k][:, m * 128:(m + 1) * 128],
                                xs_t[k][:], start=(k == 0), stop=(k == KT - 1))
                    for m in range(3):
                        nc.vector.tensor_copy(
                            mtiles[m][:, nch * 512:(nch + 1) * 512],
                            psums[m][:])
                    qkv_transposes(nch, trp, "tps")

            # loads needed by the f@W fillers / normalization, queued behind
            # the phase-Q traffic but ahead of the deferred-qkv x chunks
            bias_sb = pp.tile([128, 8], F32)
            nc.sync.dma_start(bias_sb[:], bprojT_ext[:])
            fT_sb = []
            for t in range(KT):
                ft = pp.tile([128, 512], BF16, name=f"fTsb{t}")
                nc.sync.dma_start(ft[:], fT_ext[t * 128:(t + 1) * 128, :])
                fT_sb.append(ft)

            # ---------------- phase A: attention + fillers ----------------
            with ExitStack() as actx:
                expp = actx.enter_context(tc.tile_pool(name="exp", bufs=4))
                avup = actx.enter_context(tc.tile_pool(name="avu", bufs=3))
                normp = actx.enter_context(tc.tile_pool(name="norm", bufs=2))
                avnp = actx.enter_context(tc.tile_pool(name="avn", bufs=2))
                avn8p = actx.enter_context(tc.tile_pool(name="avn8", bufs=2))
                oaccp = actx.enter_context(tc.tile_pool(name="oacc", bufs=1))
                wpp = actx.enter_context(tc.tile_pool(name="wp", bufs=1))
                wp8p = actx.enter_context(tc.tile_pool(name="wp8", bufs=1))
                rhs8p = actx.enter_context(tc.tile_pool(name="rhs8", bufs=1))
                sps = actx.enter_context(
                    tc.tile_pool(name="scores_ps", bufs=2, space="PSUM"))
                avps = actx.enter_context(
                    tc.tile_pool(name="av_ps", bufs=1, space="PSUM"))
                bcps = actx.enter_context(
                    tc.tile_pool(name="bc_ps", bufs=1, space="PSUM"))
                pjps = actx.enter_context(
                    tc.tile_pool(name="pj_ps", bufs=1, space="PSUM"))
                out_acc = [oaccp.tile([128, 512], F32, name=f"oacc{m}")
                           for m in range(8)]
                # prefetch projection weights during early attention
                wp_sb = []
                for t in range(KT):
                    w = wpp.tile([128, C], BF16, name=f"wp_{t}")
                    nc.sync.dma_start(w[:], wproj_ext[t * 128:(t + 1) * 128, :])
                    wp_sb.append(w)
                wp8_sb = []
                for hh in range(2):
                    w8 = wp8p.tile([128, 4, C], F8, name=f"wp8_{hh}")
                    nc.sync.dma_start(w8[:], wp8_ext[hh][:])
                    wp8_sb.append(w8)

                qkv_work = [(nch, m) for nch in range(2, 8) for m in range(3)]
                fw_work = list(range(8))
                pe_work = []   # 4-m-tile DoubleRow units for the h0 half
                rhs8_sb = {}

                qkv_xs_cache = {}

                def qkv_deferred_unit(unit):
                    """One m-pass of a deferred qkv chunk — small PE
                    filler emitted inside head 0's early attention chunks
                    (single pj psum slot)."""
                    nch, m = unit
                    if nch not in qkv_xs_cache:
                        qkv_xs_cache[nch] = qkv_xs(nch)
                    xs_t = qkv_xs_cache[nch]
                    pjt = pjps.tile([128, 512], F32, name="qkvd", tag="pj")
                    for k in range(KT):
                        nc.tensor.matmul(
                            pjt[:], wq_sb[k][:, m * 128:(m + 1) * 128],
                            xs_t[k][:], start=(k == 0), stop=(k == KT - 1))
                    nc.vector.tensor_copy(
                        mtiles[m][:, nch * 512:(nch + 1) * 512], pjt[:])
                    if m == 2:
                        qkv_transposes(nch, bcps, "bc")
                        del qkv_xs_cache[nch]

                def fw_m(m):
                    """f @ W_proj m-pass: dependency-free bf16 PE filler."""
                    pj = pjps.tile([128, 512], F32, name="pjf", tag="pj")
                    for t in range(KT):
                        nc.tensor.matmul(
                            pj[:], wp_sb[t][:, m * 128:(m + 1) * 128],
                            fT_sb[t][:], start=(t == 0), stop=(t == KT - 1))
                    nc.vector.tensor_scalar_add(
                        out_acc[m][:], pj[:], bias_sb[:, m:m + 1])

                def load_rhs8(hh):
                    """Stack the 8 received [64,512] bf16 tiles of half hh
                    into two DoubleRow rhs tiles [128, 2, 512] and cast to
                    fp8 on DVE (values are already x32-scaled).

                    Emit only once the collective is certainly complete:
                    these DMAs carry a wait on the a2a output and would
                    otherwise stall the sync DMA queue for everything
                    emitted after them."""
                    tiles = []
                    for u in range(2):
                        rb = rhs8p.tile([128, 2, 512], BF16,
                                        name=f"rb_{hh}{u}")
                        for i in range(2):
                            s = 2 * u + i
                            nc.sync.dma_start(
                                rb[0:64, i, :],
                                a2a_out[hh][(2 * s) * 64:(2 * s + 1) * 64, :])
                            nc.sync.dma_start(
                                rb[64:128, i, :],
                                a2a_out[hh][(2 * s + 1) * 64:(2 * s + 2) * 64, :])
                        r = rhs8p.tile([128, 2, 512], F8, name=f"r8_{hh}{u}")
                        nc.vector.tensor_copy(r[:], rb[:])
                        tiles.append(r)
                    rhs8_sb[hh] = tiles

                def proj8_unit(hh, ms, final):
                    """DoubleRow avn@W for head-parity half hh, m-tiles ms."""
                    if hh not in rhs8_sb:
                        load_rhs8(hh)
                    r8 = rhs8_sb[hh]
                    w8 = wp8_sb[hh]
                    for m in ms:
                        pj = pjps.tile([128, 512], F32, name="pj8", tag="pj")
                        for u in range(2):
                            nc.tensor.matmul(
                                pj[:], w8[:, 2 * u:2 * u + 2,
                                          m * 128:(m + 1) * 128],
                                r8[u][:], start=(u == 0), stop=(u == 1),
                                perf_mode=DR)
                        if final:
                            ot = avnp.tile([128, 512], BF16, name="ot",
                                           tag="ot")
                            nc.vector.tensor_tensor(
                                ot[:], pj[:], out_acc[m][:],
                                mybir.AluOpType.add)
                            nc.sync.dma_start(
                                out_t_ext[m * 128:(m + 1) * 128, :], ot[:])
                        else:
                            nc.vector.tensor_tensor(
                                out_acc[m][:], pj[:], out_acc[m][:],
                                mybir.AluOpType.add)

                def norm_pre(avu):
                    """1/denom chain — latency starts at chunk end."""
                    dn = normp.tile([1, 1024], F32, name="dn", tag="dn")
                    nc.sync.dma_start(dn[:], avu[64:65, :])
                    dninv = normp.tile([1, 1024], F32, name="dninv",
                                       tag="dninv")
                    nc.vector.reciprocal_approx_fast(dninv[:], dn[:])
                    dinvb = normp.tile([1, 1024], BF16, name="dinvb",
                                       tag="dinvb")
                    nc.scalar.activation(dinvb[:], dninv[:], COPY)
                    return dinvb

                def norm_chunk(h, b, ch, avu, dinvb):
                    """avn = avu[0:64]/avu[64]; attn_t out; fp8 a2a staging."""
                    po = h * 64
                    cs = b * 2048 + ch * 1024
                    avn = avnp.tile([64, 1024], BF16, name="avn", tag="avn")
                    for s in range(2):
                        bc = bcps.tile([64, 512], F32, name="bc", tag="bc")
                        nc.tensor.matmul(bc[:], ones64b[:],
                                         dinvb[:, s * 512:(s + 1) * 512],
                                         start=True, stop=True)
                        nc.vector.tensor_tensor(
                            avn[:, s * 512:(s + 1) * 512],
                            avu[0:64, s * 512:(s + 1) * 512],
                            bc[:], mybir.AluOpType.mult)
                    nc.sync.dma_start(
                        attn_t_ext[po:po + 64, cs:cs + 1024], avn[:])
                    avn32 = avn8p.tile([64, 1024], BF16, name="avn32",
                                       tag="avn32")
                    nc.vector.tensor_scalar_mul(avn32[:], avn[:], PS)
                    for j in (cs // 512, cs // 512 + 1):
                        nc.sync.dma_start(
                            a2a_in[h][j * 64:(j + 1) * 64, :],
                            avn32[:, (j * 512 - cs):(j * 512 - cs) + 512])

                def mm1_kj(h, b, cs, kj, scores_q):
                    po = h * 64
                    jt = b * 16 + kj
                    sc = sps.tile([128, 1024], F32, name="scores", tag="sc")
                    for s in range(2):
                        nc.tensor.matmul(
                            sc[:, s * 512:(s + 1) * 512],
                            kT[po:po + 64, jt * 128:(jt + 1) * 128],
                            qT[po:po + 64, cs + s * 512:cs + (s + 1) * 512],
                            start=True, stop=True)
                    scores_q[kj] = sc

                def fire_a2a(h):
                    nc.gpsimd.collective_compute(
                        "AllToAll", mybir.AluOpType.bypass,
                        replica_groups=groups,
                        ins=[a2a_in[h].opt()], outs=[a2a_out[h].opt()])
                    if h == 0:
                        pe_work.extend([(0, range(0, 4)), (0, range(4, 8))])

                def do_chunk(ci, h, b, ch, pending):
                    cs = b * 2048 + ch * 1024
                    av = avps.tile([128, 1024], F32, name="av", tag="av")
                    scores_q = {}
                    mm1_kj(h, b, cs, 0, scores_q)
                    for kj in range(16):
                        if kj + 1 < 16:
                            mm1_kj(h, b, cs, kj + 1, scores_q)
                        if kj == 4 and pending is not None:
                            ph = pending[0]
                            norm_chunk(*pending)
                            pending = None
                            if ph != h:
                                # that was the previous head's last chunk:
                                # its AllToAll half can fire now
                                fire_a2a(ph)
                        if ci < 3 and kj in (1, 3, 5) and qkv_work:
                            qkv_deferred_unit(qkv_work.pop(0))
                            if qkv_work:
                                qkv_deferred_unit(qkv_work.pop(0))
                        # 2 f@W units stay in reserve for the post-loop
                        # region (PE cover while the collectives land)
                        if 3 <= ci < 6 and kj in (1, 9) and fw_work:
                            fw_m(fw_work.pop(0))
                        sc = scores_q.pop(kj)
                        ex = expp.tile([128, 1024], BF16, name="ex", tag="ex")
                        nc.scalar.activation(ex[:], sc[:], EXP, scale=SCALE)
                        jt = b * 16 + kj
                        for s in range(2):
                            nc.tensor.matmul(
                                av[0:65, s * 512:(s + 1) * 512],
                                v_aug[h][jt][:],
                                ex[:, s * 512:(s + 1) * 512],
                                start=(kj == 0), stop=(kj == 15))
                    avu = avup.tile([65, 1024], F32, name="avu", tag="avu")
                    nc.vector.tensor_copy(avu[:], av[0:65, :])
                    return (h, b, ch, avu, norm_pre(avu))

                pending = None
                ci = 0
                for h in range(2):
                    for b in range(2):
                        for ch in range(2):
                            pending = do_chunk(ci, h, b, ch, pending)
                            ci += 1
                # final chunk of head 1, then its collective
                norm_chunk(*pending)
                fire_a2a(1)

                # reserve f@W units cover the PE while the a2a(1) staging
                # and transfer complete; only then emit the a2a-gated DMAs
                # (they park the sync queue on the collective semaphore)
                while fw_work:
                    fw_m(fw_work.pop(0))
                while pe_work:
                    proj8_unit(*pe_work.pop(0), final=False)

                # tail: head-parity-1 DoubleRow contraction + output
                proj8_unit(1, range(8), final=True)

    nc.compile()
    return nc


def kernel(x, f, W_qkv, W_proj, b_proj):
    import ml_dtypes
    from concourse.bass_utils import run_bass_kernel_spmd

    global _cached
    if _cached is None:
        _cached = _build()
    nc = _cached

    BF = ml_dtypes.bfloat16
    F8 = ml_dtypes.float8_e4m3
    x = np.ascontiguousarray(np.asarray(x, dtype=np.float32))
    f = np.ascontiguousarray(np.asarray(f, dtype=np.float32))
    W_qkv = np.asarray(W_qkv, dtype=np.float32)
    W_proj = np.asarray(W_proj, dtype=np.float32)
    b_proj = np.asarray(b_proj, dtype=np.float32)

    Wq, Wk, Wv = W_qkv[:, 0:C], W_qkv[:, C:2 * C], W_qkv[:, 2 * C:3 * C]
    xT = np.ascontiguousarray(x.reshape(BN, C).T.astype(BF))
    fT = (f.reshape(BN, C).T * PS).astype(BF)
    wproj_b = np.ascontiguousarray((W_proj * PS).astype(BF))
    bprojT = np.ascontiguousarray((b_proj * PS * PS).reshape(8, 128).T)

    # DoubleRow fp8 W_proj halves: wp8_h[p, s, m] = (W_proj*PS)[row, m] with
    # row = head*64 + p%64, head = 4s + h + 2*(p//64)
    Wp32 = W_proj * PS
    p = np.arange(128)
    wp8 = []
    for h in range(2):
        rows = np.empty((128, 4), np.int64)
        for s in range(4):
            head = 4 * s + h + 2 * (p // 64)
            rows[:, s] = head * 64 + (p % 64)
        wp8.append(np.ascontiguousarray(
            Wp32[rows, :].astype(F8).reshape(128, 4 * C)))

    in_maps = []
    for c in range(N_CORES):
        cols = slice(c * 128, (c + 1) * 128)     # heads 2c, 2c+1
        wkqv = np.ascontiguousarray(np.concatenate(
            [Wk[:, cols], Wq[:, cols], Wv[:, cols]], axis=1).astype(BF))
        in_maps.append({
            "xT": xT,
            "wkqv": wkqv,
            "fT": np.ascontiguousarray(fT[:, c * 512:(c + 1) * 512]),
            "wproj": wproj_b,
            "wp8_0": wp8[0],
            "wp8_1": wp8[1],
            "bprojT": bprojT,
        })

    res = run_bass_kernel_spmd(nc, in_maps, core_ids=list(range(N_CORES)))

    attn = np.empty((BN, C), dtype=np.float32)
    out = np.empty((BN, C), dtype=np.float32)
    for c in range(N_CORES):
        r = res.results[c]
        attn[:, c * 128:(c + 1) * 128] = r["attn_t"].T.astype(np.float32)
        out[c * 512:(c + 1) * 512, :] = \
            r["out_t"].T.astype(np.float32) / (PS * PS)
    return out.reshape(B, N, C), attn.reshape(B, N, C)


# revision 11
# speedup vs baseline: 1.4123x; 1.0713x over previous
"""Distributed MHA kernel for Trainium2 (8 NeuronCores).

Problem: x,f:(2,2048,1024), W_qkv:(1024,3072), W_proj:(1024,1024), H=16 heads.
reference returns (out, attn2gcn) with
  attn2gcn = softmax(q k^T / sqrt(64)) v   (per head, concat over heads)
  out      = (attn2gcn + f) @ W_proj + b_proj

Sharding: tensor-parallel over heads — core c owns heads 2c, 2c+1 for both
batches (column block c*128 of the hidden dim).  Attention arithmetic is
bf16 matmuls with fp32 PSUM (the attn output's max-abs rel-err budget is
too tight for fp8 anywhere on that path); softmax stays fp32 on ACT.

The projection is split: out = f @ W_proj + attn2gcn @ W_proj + b.
  - f @ W_proj runs in bf16 against host-staged fT/W tiles.  It has no
    on-device dependencies at all, so its 8 m-passes are the PE filler
    that keeps the tensor engine hot through the collective windows
    (the HAM governor halves the PE clock after any idle window).
  - attn2gcn (avn) rides the AllToAll as bf16 (x32; fp8 collectives
    measured pathologically slow), is cast to fp8e4 after the reshard,
    and the contraction runs as fp8 DoubleRow matmuls: per head-parity
    half just 2 instructions per m-tile (contraction 2x128 per instr at
    0.5 cycles/row), so the work after the last AllToAll is ~4k PE
    cycles instead of the old 33k.
  - scales: f, W_proj staged x32 (bf16), avn x32 (fp8) -> psum carries
    1024*out; the host divides by 1024 after gathering (pure numpy).

Per-core dataflow (as in the tuned baseline): qkvT = W-slice^T @ x^T in
head-packed tiles; batch-1 qkv emitted inside head 0's early attention
chunks as PE filler; attention per (head, batch) in scoresT layout,
software-pipelined so the PE runs ahead of the ACT exp; av^T accumulates
an all-ones 65th v column giving the softmax denominator for free;
normalization for chunk c-1 is emitted inside chunk c off the PE
critical path.  The head-0 AllToAll half fires mid-kernel; its DoubleRow
contraction and the f@W fillers interleave into later chunks; the head-1
half completes at the end with only the tiny DoubleRow tail behind it.
"""

import numpy as np

B, N, C, H, D = 2, 2048, 1024, 16, 64
BN = B * N
SCALE = D ** -0.5
N_CORES = 8
KT = C // 128      # 8 contraction tiles
NCH = BN // 512    # 8 qkv free chunks
PS = 32.0          # fp8/bf16 staging scale for the projection operands

_cached = None


def _build():
    from contextlib import ExitStack

    import concourse.mybir as mybir
    import concourse.tile as tile
    from concourse import bacc
    from concourse.masks import make_identity

    F32 = mybir.dt.float32
    BF16 = mybir.dt.bfloat16
    F8 = mybir.dt.float8e4
    EXP = mybir.ActivationFunctionType.Exp
    COPY = mybir.ActivationFunctionType.Copy
    DR = mybir.MatmulPerfMode.DoubleRow

    nc = bacc.Bacc("TRN2", target_bir_lowering=False, debug=False,
                   num_devices=N_CORES)

    xT_ext = nc.dram_tensor("xT", [C, BN], BF16, kind="ExternalInput").ap()
    wkqv_ext = nc.dram_tensor("wkqv", [C, 384], BF16, kind="ExternalInput").ap()
    fT_ext = nc.dram_tensor("fT", [C, 512], BF16, kind="ExternalInput").ap()
    wproj_ext = nc.dram_tensor("wproj", [C, C], BF16, kind="ExternalInput").ap()
    wp8_ext = [nc.dram_tensor(f"wp8_{h}", [128, 4 * C], mybir.dt.float8e4,
                              kind="ExternalInput").ap() for h in range(2)]
    bprojT_ext = nc.dram_tensor("bprojT", [128, 8], F32, kind="ExternalInput").ap()
    attn_t_ext = nc.dram_tensor("attn_t", [128, BN], BF16, kind="ExternalOutput").ap()
    out_t_ext = nc.dram_tensor("out_t", [C, 512], BF16, kind="ExternalOutput").ap()

    groups = [list(range(N_CORES))]

    with tile.TileContext(nc) as tc:
        with ExitStack() as octx:
            pp = octx.enter_context(tc.tile_pool(name="persist", bufs=1))
            kqp = octx.enter_context(tc.tile_pool(name="kq", bufs=1))
            vap = octx.enter_context(tc.tile_pool(name="vaug", bufs=1))
            vtp = octx.enter_context(tc.tile_pool(name="vt", bufs=1))
            wqp = octx.enter_context(tc.tile_pool(name="wq", bufs=1))
            xsp = octx.enter_context(tc.tile_pool(name="xs", bufs=16))
            dram = octx.enter_context(
                tc.tile_pool(name="dram", bufs=1, space="DRAM"))
            # qkv weights first in the DMA queues — they gate the first matmul
            wq_sb = []
            for k in range(KT):
                w = wqp.tile([128, 384], BF16, name=f"wq{k}")
                nc.sync.dma_start(w[:], wkqv_ext[k * 128:(k + 1) * 128, :])
                wq_sb.append(w)

            ident = pp.tile([128, 128], BF16)
            make_identity(nc, ident[:])
            ones64b = pp.tile([1, 64], BF16)
            nc.vector.memset(ones64b[:], 1.0)

            kT = kqp.tile([128, BN], BF16, name="kT")
            qT = kqp.tile([128, BN], BF16, name="qT")
            vT = vtp.tile([128, BN], BF16, name="vT")
            mtiles = [kT, qT, vT]
            v_aug = [[vap.tile([128, 65], BF16, name=f"va{h}_{j}")
                      for j in range(32)] for h in range(2)]

            a2a_in = [dram.tile([512, 512], BF16, name=f"a2ain{hh}")
                      for hh in range(2)]
            a2a_out = [dram.tile([512, 512], BF16, name=f"a2aout{hh}")
                       for hh in range(2)]

            def qkv_transposes(nch, tpool, ttag):
                for j in range(4 * nch, 4 * nch + 4):
                    tps = tpool.tile([128, 128], BF16, name="tps", tag=ttag)
                    nc.tensor.transpose(
                        tps[:], vT[:, j * 128:(j + 1) * 128], ident[:])
                    for h in range(2):
                        nc.vector.tensor_copy(
                            v_aug[h][j][:, 0:64], tps[:, h * 64:(h + 1) * 64])
                        nc.vector.memset(v_aug[h][j][:, 64:65], 1.0)

            def qkv_xs(nch):
                xs_t = []
                for k in range(KT):
                    xs = xsp.tile([128, 512], BF16, name="xs", tag="xs")
                    nc.sync.dma_start(
                        xs[:], xT_ext[k * 128:(k + 1) * 128,
                                      nch * 512:(nch + 1) * 512])
                    xs_t.append(xs)
                return xs_t

            # ------------- phase Q: qkv for the first two chunks -------------
            with ExitStack() as qctx:
                qps = qctx.enter_context(
                    tc.tile_pool(name="qkv_ps", bufs=1, space="PSUM"))
                trp = qctx.enter_context(
                    tc.tile_pool(name="tr_ps", bufs=2, space="PSUM"))
                for nch in range(2):
                    xs_t = qkv_xs(nch)
                    psums = [qps.tile([128, 512], F32, name=f"qps{m}",
                                      tag=f"qps{m}") for m in range(3)]
                    for k in range(KT):
                        for m in range(3):
                            nc.tensor.matmul(
                                psums[m][:],
                                wq_sb[k][:, m * 128:(m + 1) * 128],
                                xs_t[k][:], start=(k == 0), stop=(k == KT - 1))
                    for m in range(3):
                        nc.vector.tensor_copy(
                            mtiles[m][:, nch * 512:(nch + 1) * 512],
                            psums[m][:])
                    qkv_transposes(nch, trp, "tps")

            # loads needed by the f@W fillers / normalization, queued behind
            # the phase-Q traffic but ahead of the deferred-qkv x chunks
            bias_sb = pp.tile([128, 8], F32)
            nc.sync.dma_start(bias_sb[:], bprojT_ext[:])
            fT_sb = []
            for t in range(KT):
                ft = pp.tile([128, 512], BF16, name=f"fTsb{t}")
                nc.sync.dma_start(ft[:], fT_ext[t * 128:(t + 1) * 128, :])
                fT_sb.append(ft)

            # ---------------- phase A: attention + fillers ----------------
            with ExitStack() as actx:
                expp = actx.enter_context(tc.tile_pool(name="exp", bufs=4))
                avup = actx.enter_context(tc.tile_pool(name="avu", bufs=3))
                normp = actx.enter_context(tc.tile_pool(name="norm", bufs=2))
                avnp = actx.enter_context(tc.tile_pool(name="avn", bufs=2))
                avn8p = actx.enter_context(tc.tile_pool(name="avn8", bufs=2))
                oaccp = actx.enter_context(tc.tile_pool(name="oacc", bufs=1))
                wpp = actx.enter_context(tc.tile_pool(name="wp", bufs=1))
                wp8p = actx.enter_context(tc.tile_pool(name="wp8", bufs=1))
                rhs8p = actx.enter_context(tc.tile_pool(name="rhs8", bufs=1))
                sps = actx.enter_context(
                    tc.tile_pool(name="scores_ps", bufs=2, space="PSUM"))
                avps = actx.enter_context(
                    tc.tile_pool(name="av_ps", bufs=1, space="PSUM"))
                bcps = actx.enter_context(
                    tc.tile_pool(name="bc_ps", bufs=1, space="PSUM"))
                pjps = actx.enter_context(
                    tc.tile_pool(name="pj_ps", bufs=1, space="PSUM"))
                out_acc = [oaccp.tile([128, 512], F32, name=f"oacc{m}")
                           for m in range(8)]
                # prefetch projection weights during early attention
                wp_sb = []
                for t in range(KT):
                    w = wpp.tile([128, C], BF16, name=f"wp_{t}")
                    nc.sync.dma_start(w[:], wproj_ext[t * 128:(t + 1) * 128, :])
                    wp_sb.append(w)
                wp8_sb = []
                for hh in range(2):
                    w8 = wp8p.tile([128, 4, C], F8, name=f"wp8_{hh}")
                    nc.sync.dma_start(w8[:], wp8_ext[hh][:])
                    wp8_sb.append(w8)

                qkv_work = [(nch, m) for nch in range(2, 8) for m in range(3)]
                fw_work = list(range(8))
                pe_work = []   # 4-m-tile DoubleRow units for the h0 half
                rhs8_sb = {}

                qkv_xs_cache = {}

                def qkv_deferred_unit(unit):
                    """One m-pass of a deferred qkv chunk — small PE
                    filler emitted inside head 0's early attention chunks
                    (single pj psum slot)."""
                    nch, m = unit
                    if nch not in qkv_xs_cache:
                        qkv_xs_cache[nch] = qkv_xs(nch)
                    xs_t = qkv_xs_cache[nch]
                    pjt = pjps.tile([128, 512], F32, name="qkvd", tag="pj")
                    for k in range(KT):
                        nc.tensor.matmul(
                            pjt[:], wq_sb[k][:, m * 128:(m + 1) * 128],
                            xs_t[k][:], start=(k == 0), stop=(k == KT - 1))
                    nc.vector.tensor_copy(
                        mtiles[m][:, nch * 512:(nch + 1) * 512], pjt[:])
                    if m == 2:
                        qkv_transposes(nch, bcps, "bc")
                        del qkv_xs_cache[nch]

                def fw_m(m):
                    """f @ W_proj m-pass: dependency-free bf16 PE filler."""
                    pj = pjps.tile([128, 512], F32, name="pjf", tag="pj")
                    for t in range(KT):
                        nc.tensor.matmul(
                            pj[:], wp_sb[t][:, m * 128:(m + 1) * 128],
                            fT_sb[t][:], start=(t == 0), stop=(t == KT - 1))
                    nc.vector.tensor_scalar_add(
                        out_acc[m][:], pj[:], bias_sb[:, m:m + 1])

                def load_rhs8(hh):
                    """Stack the 8 received [64,512] bf16 tiles of half hh
                    into two DoubleRow rhs tiles [128, 2, 512] and cast to
                    fp8 on DVE (values are already x32-scaled).

                    Emit only once the collective is certainly complete:
                    these DMAs carry a wait on the a2a output and would
                    otherwise stall the sync DMA queue for everything
                    emitted after them."""
                    tiles = []
                    for u in range(2):
                        rb = rhs8p.tile([128, 2, 512], BF16,
                                        name=f"rb_{hh}{u}")
                        for i in range(2):
                            s = 2 * u + i
                            nc.sync.dma_start(
                                rb[0:64, i, :],
                                a2a_out[hh][(2 * s) * 64:(2 * s + 1) * 64, :])
                            nc.sync.dma_start(
                                rb[64:128, i, :],
                                a2a_out[hh][(2 * s + 1) * 64:(2 * s + 2) * 64, :])
                        r = rhs8p.tile([128, 2, 512], F8, name=f"r8_{hh}{u}")
                        nc.vector.tensor_copy(r[:], rb[:])
                        tiles.append(r)
                    rhs8_sb[hh] = tiles

                def proj8_unit(hh, ms, final):
                    """DoubleRow avn@W for head-parity half hh, m-tiles ms."""
                    if hh not in rhs8_sb:
                        load_rhs8(hh)
                    r8 = rhs8_sb[hh]
                    w8 = wp8_sb[hh]
                    for m in ms:
                        pj = pjps.tile([128, 512], F32, name="pj8", tag="pj")
                        for u in range(2):
                            nc.tensor.matmul(
                                pj[:], w8[:, 2 * u:2 * u + 2,
                                          m * 128:(m + 1) * 128],
                                r8[u][:], start=(u == 0), stop=(u == 1),
                                perf_mode=DR)
                        if final:
                            ot = avnp.tile([128, 512], BF16, name="ot",
                                           tag="ot")
                            nc.vector.tensor_tensor(
                                ot[:], pj[:], out_acc[m][:],
                                mybir.AluOpType.add)
                            nc.sync.dma_start(
                                out_t_ext[m * 128:(m + 1) * 128, :], ot[:])
                        else:
                            nc.vector.tensor_tensor(
                                out_acc[m][:], pj[:], out_acc[m][:],
                                mybir.AluOpType.add)

                def norm_pre(avu):
                    """1/denom chain — latency starts at chunk end."""
                    dn = normp.tile([1, 1024], F32, name="dn", tag="dn")
                    nc.sync.dma_start(dn[:], avu[64:65, :])
                    dninv = normp.tile([1, 1024], F32, name="dninv",
                                       tag="dninv")
                    nc.vector.reciprocal_approx_fast(dninv[:], dn[:])
                    dinvb = normp.tile([1, 1024], BF16, name="dinvb",
                                       tag="dinvb")
                    # on DVE, not ACT: an ACT copy would make every next
                    # chunk's exps queue behind this chain (ACT is in-order)
                    nc.vector.tensor_copy(dinvb[:], dninv[:])
                    return dinvb

                def norm_chunk(h, b, ch, avu, dinvb):
                    """avn = avu[0:64]/avu[64]; attn_t out; fp8 a2a staging."""
                    po = h * 64
                    cs = b * 2048 + ch * 1024
                    avn = avnp.tile([64, 1024], BF16, name="avn", tag="avn")
                    for s in range(2):
                        bc = bcps.tile([64, 512], F32, name="bc", tag="bc")
                        nc.tensor.matmul(bc[:], ones64b[:],
                                         dinvb[:, s * 512:(s + 1) * 512],
                                         start=True, stop=True)
                        nc.vector.tensor_tensor(
                            avn[:, s * 512:(s + 1) * 512],
                            avu[0:64, s * 512:(s + 1) * 512],
                            bc[:], mybir.AluOpType.mult)
                    nc.sync.dma_start(
                        attn_t_ext[po:po + 64, cs:cs + 1024], avn[:])
                    avn32 = avn8p.tile([64, 1024], BF16, name="avn32",
                                       tag="avn32")
                    nc.vector.tensor_scalar_mul(avn32[:], avn[:], PS)
                    for j in (cs // 512, cs // 512 + 1):
                        nc.sync.dma_start(
                            a2a_in[h][j * 64:(j + 1) * 64, :],
                            avn32[:, (j * 512 - cs):(j * 512 - cs) + 512])

                def mm1_kj(h, b, cs, kj, scores_q):
                    po = h * 64
                    jt = b * 16 + kj
                    sc = sps.tile([128, 1024], F32, name="scores", tag="sc")
                    for s in range(2):
                        nc.tensor.matmul(
                            sc[:, s * 512:(s + 1) * 512],
                            kT[po:po + 64, jt * 128:(jt + 1) * 128],
                            qT[po:po + 64, cs + s * 512:cs + (s + 1) * 512],
                            start=True, stop=True)
                    scores_q[kj] = sc

                def fire_a2a(h):
                    nc.gpsimd.collective_compute(
                        "AllToAll", mybir.AluOpType.bypass,
                        replica_groups=groups,
                        ins=[a2a_in[h].opt()], outs=[a2a_out[h].opt()])
                    if h == 0:
                        pe_work.extend([(0, range(0, 4)), (0, range(4, 8))])

                def do_chunk(ci, h, b, ch, pending):
                    cs = b * 2048 + ch * 1024
                    av = avps.tile([128, 1024], F32, name="av", tag="av")
                    scores_q = {}
                    mm1_kj(h, b, cs, 0, scores_q)
                    for kj in range(16):
                        if kj + 1 < 16:
                            mm1_kj(h, b, cs, kj + 1, scores_q)
                        if kj == 4 and pending is not None:
                            ph = pending[0]
                            norm_chunk(*pending)
                            pending = None
                            if ph != h:
                                # that was the previous head's last chunk:
                                # its AllToAll half can fire now
                                fire_a2a(ph)
                        if ci < 3 and kj in (1, 3, 5) and qkv_work:
                            qkv_deferred_unit(qkv_work.pop(0))
                            if qkv_work:
                                qkv_deferred_unit(qkv_work.pop(0))
                        # f@W fillers: 2 per chunk mid-kernel, 1 in each of
                        # the last two chunks so the PE never idles there
                        # (an idle window trips the HAM clock governor)
                        if ((3 <= ci < 6 and kj in (1, 9))
                                or (ci >= 6 and kj == 5)) and fw_work:
                            fw_m(fw_work.pop(0))
                        sc = scores_q.pop(kj)
                        ex = expp.tile([128, 1024], BF16, name="ex", tag="ex")
                        nc.scalar.activation(ex[:], sc[:], EXP, scale=SCALE)
                        jt = b * 16 + kj
                        for s in range(2):
                            nc.tensor.matmul(
                                av[0:65, s * 512:(s + 1) * 512],
                                v_aug[h][jt][:],
                                ex[:, s * 512:(s + 1) * 512],
                                start=(kj == 0), stop=(kj == 15))
                    avu = avup.tile([65, 1024], F32, name="avu", tag="avu")
                    nc.vector.tensor_copy(avu[:], av[0:65, :])
                    return (h, b, ch, avu, norm_pre(avu))

                pending = None
                ci = 0
                for h in range(2):
                    for b in range(2):
                        for ch in range(2):
                            pending = do_chunk(ci, h, b, ch, pending)
                            ci += 1
                # final chunk of head 1, then its collective
                norm_chunk(*pending)
                fire_a2a(1)

                # reserve f@W units cover the PE while the a2a(1) staging
                # and transfer complete; only then emit the a2a-gated DMAs
                # (they park the sync queue on the collective semaphore)
                while fw_work:
                    fw_m(fw_work.pop(0))
                while pe_work:
                    proj8_unit(*pe_work.pop(0), final=False)

                # tail: head-parity-1 DoubleRow contraction + output
                proj8_unit(1, range(8), final=True)

    nc.compile()
    return nc


def kernel(x, f, W_qkv, W_proj, b_proj):
    import ml_dtypes
    from concourse.bass_utils import run_bass_kernel_spmd

    global _cached
    if _cached is None:
        _cached = _build()
    nc = _cached

    BF = ml_dtypes.bfloat16
    F8 = ml_dtypes.float8_e4m3
    x = np.ascontiguousarray(np.asarray(x, dtype=np.float32))
    f = np.ascontiguousarray(np.asarray(f, dtype=np.float32))
    W_qkv = np.asarray(W_qkv, dtype=np.float32)
    W_proj = np.asarray(W_proj, dtype=np.float32)
    b_proj = np.asarray(b_proj, dtype=np.float32)

    Wq, Wk, Wv = W_qkv[:, 0:C], W_qkv[:, C:2 * C], W_qkv[:, 2 * C:3 * C]
    xT = np.ascontiguousarray(x.reshape(BN, C).T.astype(BF))
    fT = (f.reshape(BN, C).T * PS).astype(BF)
    wproj_b = np.ascontiguousarray((W_proj * PS).astype(BF))
    bprojT = np.ascontiguousarray((b_proj * PS * PS).reshape(8, 128).T)

    # DoubleRow fp8 W_proj halves: wp8_h[p, s, m] = (W_proj*PS)[row, m] with
    # row = head*64 + p%64, head = 4s + h + 2*(p//64)
    Wp32 = W_proj * PS
    p = np.arange(128)
    wp8 = []
    for h in range(2):
        rows = np.empty((128, 4), np.int64)
        for s in range(4):
            head = 4 * s + h + 2 * (p // 64)
            rows[:, s] = head * 64 + (p % 64)
        wp8.append(np.ascontiguousarray(
            Wp32[rows, :].astype(F8).reshape(128, 4 * C)))

    in_maps = []
    for c in range(N_CORES):
        cols = slice(c * 128, (c + 1) * 128)     # heads 2c, 2c+1
        wkqv = np.ascontiguousarray(np.concatenate(
            [Wk[:, cols], Wq[:, cols], Wv[:, cols]], axis=1).astype(BF))
        in_maps.append({
            "xT": xT,
            "wkqv": wkqv,
            "fT": np.ascontiguousarray(fT[:, c * 512:(c + 1) * 512]),
            "wproj": wproj_b,
            "wp8_0": wp8[0],
            "wp8_1": wp8[1],
            "bprojT": bprojT,
        })

    res = run_bass_kernel_spmd(nc, in_maps, core_ids=list(range(N_CORES)))

    attn = np.empty((BN, C), dtype=np.float32)
    out = np.empty((BN, C), dtype=np.float32)
    for c in range(N_CORES):
        r = res.results[c]
        attn[:, c * 128:(c + 1) * 128] = r["attn_t"].T.astype(np.float32)
        out[c * 512:(c + 1) * 512, :] = \
            r["out_t"].T.astype(np.float32) / (PS * PS)
    return out.reshape(B, N, C), attn.reshape(B, N, C)
